# revision 12
# baseline (speedup 1.0000x reference)
"""Trainium2 Bass kernel for nn_DotProductAttention_50010599194781.

Computes, per (batch*head) bh:
    S = Q @ K^T / sqrt(d)                       [L, L]
    P = softmax(mask(S))                         (mask: key index >= valid_lens[bh] -> -1e6)
    mult = (q_mult[b] @ kv_mult[b]^T) / (||.||_F + 1e-5)   (per batch b, repeated over heads)
    out = (P + mult) @ V

Sharding: BH=32 heads split 4-per-core across 8 NeuronCores (SPMD, one program).

Device algorithm (per core, batch b fixed):
  - mult term via associativity:  mult @ V_h = q_mult @ (kv_mult^T @ V_h) / fro
    and  fro^2 = sum(G_q * G_k)  with  G_q = q_mult^T q_mult,  G_k = kv_mult^T kv_mult.
  - softmax part computed in transposed layout S^T[k, q] so exp(S^T) tiles are directly
    usable as matmul stationary weights for P @ V (no on-chip transposes).
  - masking is pure host-side data: V is pre-masked per head and an appended ones-column
    (also masked) yields the softmax denominator for free inside the P@V matmul.
    valid_len == 0 heads get scale=0 (exp->1 uniform) + unmasked V, matching jax softmax.
"""

import numpy as np
import ml_dtypes

import concourse.bass as bass
import concourse.tile as tile
from concourse import bacc, mybir, bass_utils
from concourse.tile_rust import add_dep_helper

B, H, L, D = 2, 16, 2048, 128
NCORES = 8
HPC = 4            # heads per core
NT = L // 128      # 16 k-tiles of 128
NJB = 4            # q-blocks of 512
PADC = 132         # PV rhs cols: 128 V + 1 ones + 3 pad (bf16)

f32 = mybir.dt.float32
f32r = mybir.dt.float32r
bf16 = mybir.dt.bfloat16

MULT = mybir.AluOpType.mult
ADD = mybir.AluOpType.add
EXP = mybir.ActivationFunctionType.Exp
LN = mybir.ActivationFunctionType.Ln


def _build_body(nc, tc, kt_d, qt_d, vx_d, v4_d, cm_d, qmt_d, sc_d, out_d, tvs, cfg):
    with tc.tile_pool(name="pers", bufs=1) as pers:
        tsx_sb = pers.tile([128, HPC, 256], bf16)
        scales_sb = pers.tile([128, HPC], f32)
        qmt_sb = pers.tile([128, L], bf16)
        nc.sync.dma_start(out=scales_sb, in_=sc_d)
        nc.sync.dma_start(out=qmt_sb, in_=qmt_d)
        ones_col = pers.tile([128, 1], f32)
        nc.vector.memset(ones_col, 1.0)
        ones_row = pers.tile([1, 128], f32)
        nc.vector.memset(ones_row, 1.0)

        # ---------------- phase 0: mult-term prep (shared by the 4 heads) ---------
        with tc.tile_pool(name="p0", bufs=1) as p0, \
             tc.tile_pool(name="pp0", bufs=1, space="PSUM") as pp0:
            cm_sb = p0.tile([128, NT, 256], f32r)
            nc.sync.dma_start(out=cm_sb, in_=cm_d.rearrange("p (t c) -> p t c", t=NT))
            v4_sb = p0.tile([128, NT, 512], f32r)
            nc.sync.dma_start(out=v4_sb, in_=v4_d.rearrange("p (t c) -> p t c", t=NT))

            g1 = pp0.tile([128, 256], f32)    # q_mult^T @ [q_mult | kv_mult]
            g2 = pp0.tile([128, 256], f32)    # kv_mult^T @ [q_mult | kv_mult]
            tps = pp0.tile([128, 512], f32)   # kv_mult^T @ [V_h0 .. V_h3]
            for t in range(NT):
                nc.tensor.matmul(g1, lhsT=cm_sb[:, t, 0:128],
                                 rhs=cm_sb[:, t, :],
                                 start=(t == 0), stop=(t == NT - 1))
            for t in range(NT):
                nc.tensor.matmul(g2, lhsT=cm_sb[:, t, 128:256],
                                 rhs=cm_sb[:, t, :],
                                 start=(t == 0), stop=(t == NT - 1))
            for t in range(NT):
                nc.tensor.matmul(tps, lhsT=cm_sb[:, t, 128:256],
                                 rhs=v4_sb[:, t, :],
                                 start=(t == 0), stop=(t == NT - 1))

            # fro^2 = sum(G_q * G_k); DVE reads one PSUM operand per op.
            gq_sb = p0.tile([128, 128], f32)
            nc.vector.tensor_copy(gq_sb, g1[:, 0:128])
            prod = p0.tile([128, 128], f32)
            rowsum = p0.tile([128, 1], f32)
            nc.vector.scalar_tensor_tensor(out=prod, in0=gq_sb, scalar=1.0,
                                           in1=g2[:, 128:256], op0=MULT, op1=MULT,
                                           accum_out=rowsum)
            fro2p = pp0.tile([1, 1], f32)
            nc.tensor.matmul(fro2p, lhsT=rowsum,
                             rhs=ones_col, start=True, stop=True)
            fro2_sb = p0.tile([1, 1], f32)
            nc.scalar.copy(fro2_sb, fro2p)
            frob = pp0.tile([128, 1], f32)
            nc.tensor.matmul(frob, lhsT=ones_row,
                             rhs=fro2_sb, start=True, stop=True)
            # sqrt(x) = exp(0.5*ln(x)): stays in the natural_log_exp table set and
            # avoids the low-precision Sqrt spline.
            lnf = p0.tile([128, 1], f32)
            nc.scalar.activation(lnf, frob, LN)
            fro_sb = p0.tile([128, 1], f32)
            nc.scalar.activation(fro_sb, lnf, EXP, scale=0.5)
            froe = p0.tile([128, 1], f32)
            nc.vector.tensor_scalar_add(froe, fro_sb, 1e-5)
            rs_sb = p0.tile([128, 1], f32)
            nc.vector.reciprocal(rs_sb, froe)

            nc.vector.memset(tsx_sb, 0.0)
            nc.vector.tensor_scalar_mul(
                tsx_sb[:, :, 0:128],
                tps.rearrange("p (h d) -> p h d", h=HPC),
                rs_sb,
            )

        # ---------------- per-head attention ----------------
        TG = cfg[0]
        bv = cfg[3] if len(cfg) > 3 else 0
        sps_bufs = {1: 4, 2: 2, 3: 2}[TG]
        pvs_bufs, oms_bufs = (3, 1) if bv == 1 else (2, 2)
        with tc.tile_pool(name="kq", bufs=2) as kq, \
             tc.tile_pool(name="vxp", bufs=2) as vxp, \
             tc.tile_pool(name="ep", bufs=cfg[4] if len(cfg) > 4 else 3) as ep, \
             tc.tile_pool(name="ob", bufs=3) as ob, \
             tc.tile_pool(name="sm", bufs=4) as sm, \
             tc.tile_pool(name="sps", bufs=sps_bufs, space="PSUM") as sps, \
             tc.tile_pool(name="pvs", bufs=pvs_bufs, space="PSUM") as pvs, \
             tc.tile_pool(name="oms", bufs=oms_bufs, space="PSUM") as oms:
            TG, om_in_pv = cfg[0], cfg[1]
            for h in range(HPC):
                tv = tvs[h]
                s_dt = f32r if cfg[2] == 'f32r' else bf16
                kt_sb = kq.tile([128, L], s_dt, tag="kt")
                nc.sync.dma_start(out=kt_sb[:, 0:tv * 128], in_=kt_d[h][:, 0:tv * 128])
                qt_sb = kq.tile([128, L], s_dt, tag="qt")
                nc.sync.dma_start(out=qt_sb, in_=qt_d[h])
                vx_sb = vxp.tile([128, NT, PADC], bf16, tag="vx")
                nc.sync.dma_start(
                    out=vx_sb[:, 0:tv, :],
                    in_=vx_d[h].rearrange("p (t c) -> p t c", t=NT)[:, 0:tv, :])

                for jb in range(NJB):
                    # two PSUM banks, each packing two q-subtile accumulators
                    # (cols [0:PADC] and [256:256+PADC]); col 128 of each region is
                    # the softmax denominator (ones column of vx).
                    pv0 = pvs.tile([128, 512], f32, tag="pv", name="pv0")
                    pv1 = pvs.tile([128, 512], f32, tag="pv", name="pv1")
                    pvb = [pv0, pv1]
                    pv_first = [None, None]

                    for tg in range((tv + TG - 1) // TG):
                        tls = [i for i in range(TG) if TG * tg + i < tv]
                        width = 512 * len(tls)
                        sp_t = sps.tile([128, 512 * TG], f32, tag="sp", name="sp_t")
                        for tl in tls:
                            t = TG * tg + tl
                            nc.tensor.matmul(
                                sp_t[:, tl * 512:(tl + 1) * 512],
                                lhsT=kt_sb[:, t * 128:(t + 1) * 128],
                                rhs=qt_sb[:, jb * 512:(jb + 1) * 512],
                                start=True, stop=True)
                        exps = ep.tile([128, 512 * TG], bf16, tag="exps", name="exps")
                        nc.scalar.activation(exps[:, 0:width], sp_t[:, 0:width], EXP,
                                             scale=scales_sb[:, h:h + 1])
                        for tl in tls:
                            t = TG * tg + tl
                            for qs in range(4):
                                bank = pvb[qs // 2]
                                off = (qs % 2) * 256
                                # has_written packing: the first matmul emitted into a
                                # bank uses start=True (clears the whole bank's bits);
                                # the other region's t=0 matmul relies on its bits being
                                # clear -> plain write, then accumulates for t>=1.
                                st = (t == 0 and qs % 2 == 0)
                                mm = nc.tensor.matmul(
                                    bank[:, off:off + PADC],
                                    lhsT=exps[:, tl * 512 + qs * 128:
                                              tl * 512 + (qs + 1) * 128],
                                    rhs=vx_sb[:, t, :],
                                    start=st,
                                    stop=(t == tv - 1 and qs % 2 == 1),
                                    skip_group_check=True)
                                if t == 0:
                                    if qs % 2 == 0:
                                        pv_first[qs // 2] = mm
                                    else:
                                        add_dep_helper(
                                            mm.ins, pv_first[qs // 2].ins,
                                            sync=False,
                                            reason="psum has_written bank packing")

                    # epilogue part 1: rec + t1 per qs
                    t1s = []
                    for qs in range(4):
                        bank = pvb[qs // 2]
                        off = (qs % 2) * 256
                        rec = sm.tile([128, 1], f32, tag="rec", name="rec")
                        nc.vector.reciprocal(rec, bank[:, off + 128:off + 129])
                        t1 = sm.tile([128, 128], f32, tag="t1", name="t1")
                        nc.vector.tensor_scalar_mul(t1, bank[:, off:off + 128], rec)
                        t1s.append(t1)

                    # mult-term: out_m = q_mult @ (kv_mult^T V_h / fro)
                    if om_in_pv:
                        om_tiles = [(pv0, qs * 128) for qs in range(4)]
                    else:
                        om0 = oms.tile([128, 512], f32, tag="om", name="om0")
                        om_tiles = [(om0, qs * 128) for qs in range(4)]
                    om_first = None
                    for qs in range(4):
                        bank, off = om_tiles[qs]
                        mm = nc.tensor.matmul(
                            bank[:, off:off + 128],
                            lhsT=qmt_sb[:, jb * 512 + qs * 128:
                                        jb * 512 + (qs + 1) * 128],
                            rhs=tsx_sb[:, h, 0:128],
                            start=(qs == 0), stop=(qs == 3),
                            skip_group_check=True)
                        if qs == 0:
                            om_first = mm
                        else:
                            add_dep_helper(mm.ins, om_first.ins, sync=False,
                                           reason="psum has_written bank packing")

                    osb = ob.tile([128, 4, 128], f32, tag="osb", name="osb")
                    for qs in range(4):
                        bank, off = om_tiles[qs]
                        nc.vector.tensor_add(
                            osb[:, qs, :], t1s[qs], bank[:, off:off + 128])
                    nc.sync.dma_start(
                        out=out_d[h, jb * 512:(jb + 1) * 512, :]
                            .rearrange("(s p) d -> p s d", p=128),
                        in_=osb)


def build_program(repeat: int = 1, tvs=(NT,) * HPC, cfg=(2, False, 'f32r', 0)):
    nc = bacc.Bacc("TRN2", target_bir_lowering=False, debug=False,
                   enable_asserts=False, num_devices=NCORES)
    s_dt = f32r if cfg[2] == 'f32r' else bf16
    kt_d = nc.dram_tensor("kt", (HPC, 128, L), s_dt, kind="ExternalInput").ap()
    qt_d = nc.dram_tensor("qt", (HPC, 128, L), s_dt, kind="ExternalInput").ap()
    vx_d = nc.dram_tensor("vx", (HPC, 128, NT * PADC), bf16, kind="ExternalInput").ap()
    v4_d = nc.dram_tensor("v4", (128, NT * 512), f32r, kind="ExternalInput").ap()
    cm_d = nc.dram_tensor("cm", (128, NT * 256), f32r, kind="ExternalInput").ap()
    qmt_d = nc.dram_tensor("qmt", (128, L), bf16, kind="ExternalInput").ap()
    sc_d = nc.dram_tensor("sc", (128, HPC), f32, kind="ExternalInput").ap()
    out_d = nc.dram_tensor("out", (HPC, L, D), f32, kind="ExternalOutput").ap()

    with tile.TileContext(nc) as tc:
        for _ in range(repeat):
            _build_body(nc, tc, kt_d, qt_d, vx_d, v4_d, cm_d, qmt_d, sc_d, out_d, tvs, cfg)
    nc.compile()
    return nc


def head_order_and_tvs(valid_lens):
    vl = np.asarray(valid_lens).astype(np.int64)
    tv_all = np.where(vl == 0, NT, -(-vl // 128)).astype(int).reshape(NCORES, HPC)
    order = np.argsort(-tv_all, axis=1, kind="stable")
    sorted_tv = -np.sort(-tv_all, axis=1)
    tvs = tuple(int(x) for x in sorted_tv.max(axis=0))
    return order, tvs


def host_prepare(queries, keys, values, q_mult, kv_mult, valid_lens, num_heads,
                 order=None, s_bf16=False):
    queries = np.asarray(queries, dtype=np.float32)
    keys = np.asarray(keys, dtype=np.float32)
    values = np.asarray(values, dtype=np.float32)
    q_mult = np.asarray(q_mult, dtype=np.float32)
    kv_mult = np.asarray(kv_mult, dtype=np.float32)
    valid_lens = np.asarray(valid_lens).astype(np.int64)

    if order is None:
        order = np.tile(np.arange(HPC), (NCORES, 1))
    in_maps = []
    for c in range(NCORES):
        s = HPC * c
        b = s // num_heads
        idx = [s + int(order[c][i]) for i in range(HPC)]
        kt = np.ascontiguousarray(keys[idx].transpose(0, 2, 1))
        qt = np.ascontiguousarray(queries[idx].transpose(0, 2, 1))
        if s_bf16:
            kt = kt.astype(ml_dtypes.bfloat16)
            qt = qt.astype(ml_dtypes.bfloat16)

        vx = np.zeros((HPC, L, PADC), np.float32)
        sc = np.zeros((128, HPC), np.float32)
        for i in range(HPC):
            v = int(valid_lens[idx[i]])
            if v == 0:
                vx[i, :, 0:D] = values[idx[i]]
                vx[i, :, D] = 1.0
                sc[:, i] = 0.0
            else:
                m = (np.arange(L) < v).astype(np.float32)
                vx[i, :, 0:D] = values[idx[i]] * m[:, None]
                vx[i, :, D] = m
                sc[:, i] = 1.0 / np.sqrt(float(D))
        vxr = np.ascontiguousarray(
            vx.reshape(HPC, NT, 128, PADC).transpose(0, 2, 1, 3)
              .reshape(HPC, 128, NT * PADC)).astype(ml_dtypes.bfloat16)

        v4 = np.ascontiguousarray(
            values[idx].reshape(HPC, NT, 128, D).transpose(2, 1, 0, 3)
                  .reshape(128, NT * HPC * D))

        cmf = np.concatenate([q_mult[b], kv_mult[b]], axis=1)          # [L, 256]
        cm = np.ascontiguousarray(
            cmf.reshape(NT, 128, 256).transpose(1, 0, 2).reshape(128, NT * 256))
        qmt = np.ascontiguousarray(q_mult[b].T).astype(ml_dtypes.bfloat16)

        in_maps.append(dict(kt=kt, qt=qt, vx=vxr, v4=v4, cm=cm, qmt=qmt, sc=sc))
    return in_maps


_PROGRAM_CACHE = {}


DEFAULT_CFG = (2, False, 'f32r', 0, 5)   # (TG, om-in-pv, S dtype, bufs variant, ep bufs)


def _get_program(repeat: int = 1, tvs=(NT,) * HPC, cfg=None):
    cfg = DEFAULT_CFG if cfg is None else cfg
    key = (repeat, tuple(tvs), cfg)
    if key not in _PROGRAM_CACHE:
        _PROGRAM_CACHE[key] = build_program(repeat, tvs, cfg)
    return _PROGRAM_CACHE[key]


def kernel(queries, keys, values, q_mult, kv_mult, valid_lens, num_heads, **_unused):
    num_heads = int(np.asarray(num_heads))
    order, tvs = head_order_and_tvs(valid_lens)
    in_maps = host_prepare(queries, keys, values, q_mult, kv_mult, valid_lens,
                           num_heads, order)
    nc = _get_program(1, tvs)
    res = None
    for attempt in range(3):
        try:
            res = bass_utils.run_bass_kernel_spmd(
                nc, in_maps, core_ids=list(range(NCORES)))
            break
        except Exception:
            if attempt == 2:
                raise
            import time as _time
            _time.sleep(5)
    out = np.empty((NCORES * HPC, L, D), np.float32)
    for c in range(NCORES):
        o = np.asarray(res.results[c]["out"], np.float32)
        for i in range(HPC):
            out[HPC * c + int(order[c][i])] = o[i]
    return out


# revision 14
# speedup vs baseline: 1.1667x; 1.1667x over previous
"""Trainium2 Bass kernel for nn_DotProductAttention_50010599194781.

Computes, per (batch*head) bh:
    S = Q @ K^T / sqrt(d)                       [L, L]
    P = softmax(mask(S))                         (mask: key index >= valid_lens[bh] -> -1e6)
    mult = (q_mult[b] @ kv_mult[b]^T) / (||.||_F + 1e-5)   (per batch b, repeated over heads)
    out = (P + mult) @ V

Sharding: BH=32 heads split 4-per-core across 8 NeuronCores (SPMD, one program).

Device algorithm (per core, batch b fixed):
  - mult term via associativity:  mult @ V_h = q_mult @ (kv_mult^T @ V_h) / fro
    and  fro^2 = sum(G_q * G_k)  with  G_q = q_mult^T q_mult,  G_k = kv_mult^T kv_mult.
  - softmax part computed in transposed layout S^T[k, q] so exp(S^T) tiles are directly
    usable as matmul stationary weights for P @ V (no on-chip transposes).
  - masking is pure host-side data: V is pre-masked per head and an appended ones-column
    (also masked) yields the softmax denominator for free inside the P@V matmul.
    valid_len == 0 heads get scale=0 (exp->1 uniform) + unmasked V, matching jax softmax.
"""

import numpy as np
import ml_dtypes

import concourse.bass as bass
import concourse.tile as tile
from concourse import bacc, mybir, bass_utils
from concourse.tile_rust import add_dep_helper

B, H, L, D = 2, 16, 2048, 128
NCORES = 8
HPC = 4            # heads per core
NT = L // 128      # 16 k-tiles of 128
NJB = 4            # q-blocks of 512
PADC = 132         # PV rhs cols: 128 V + 1 ones + 3 pad (bf16)

f32 = mybir.dt.float32
f32r = mybir.dt.float32r
bf16 = mybir.dt.bfloat16

MULT = mybir.AluOpType.mult
ADD = mybir.AluOpType.add
EXP = mybir.ActivationFunctionType.Exp
LN = mybir.ActivationFunctionType.Ln


def _build_body(nc, tc, kt_d, qt_d, vx_d, v4_d, cm_d, qmt_d, sc_d, out_d, tvs, cfg):
    with tc.tile_pool(name="pers", bufs=1) as pers:
        tsx_sb = pers.tile([128, HPC, 256], bf16)
        scales_sb = pers.tile([128, HPC], f32)
        qmt_sb = pers.tile([128, L], bf16)
        nc.sync.dma_start(out=scales_sb, in_=sc_d)
        nc.sync.dma_start(out=qmt_sb, in_=qmt_d)
        ones_col = pers.tile([128, 1], f32)
        nc.vector.memset(ones_col, 1.0)
        ones_row = pers.tile([1, 128], f32)
        nc.vector.memset(ones_row, 1.0)

        # ---------------- phase 0: mult-term prep (shared by the 4 heads) ---------
        with tc.tile_pool(name="p0", bufs=1) as p0, \
             tc.tile_pool(name="pp0", bufs=1, space="PSUM") as pp0:
            cm_sb = p0.tile([128, NT, 256], f32r)
            nc.sync.dma_start(out=cm_sb, in_=cm_d.rearrange("p (t c) -> p t c", t=NT))
            v4_sb = p0.tile([128, NT, 512], f32r)
            nc.sync.dma_start(out=v4_sb, in_=v4_d.rearrange("p (t c) -> p t c", t=NT))

            g1 = pp0.tile([128, 256], f32)    # q_mult^T @ [q_mult | kv_mult]
            g2 = pp0.tile([128, 256], f32)    # kv_mult^T @ [q_mult | kv_mult]
            tps = pp0.tile([128, 512], f32)   # kv_mult^T @ [V_h0 .. V_h3]
            for t in range(NT):
                nc.tensor.matmul(g1, lhsT=cm_sb[:, t, 0:128],
                                 rhs=cm_sb[:, t, :],
                                 start=(t == 0), stop=(t == NT - 1))
            for t in range(NT):
                nc.tensor.matmul(g2, lhsT=cm_sb[:, t, 128:256],
                                 rhs=cm_sb[:, t, :],
                                 start=(t == 0), stop=(t == NT - 1))
            for t in range(NT):
                nc.tensor.matmul(tps, lhsT=cm_sb[:, t, 128:256],
                                 rhs=v4_sb[:, t, :],
                                 start=(t == 0), stop=(t == NT - 1))

            # fro^2 = sum(G_q * G_k); DVE reads one PSUM operand per op.
            gq_sb = p0.tile([128, 128], f32)
            nc.vector.tensor_copy(gq_sb, g1[:, 0:128])
            prod = p0.tile([128, 128], f32)
            rowsum = p0.tile([128, 1], f32)
            nc.vector.scalar_tensor_tensor(out=prod, in0=gq_sb, scalar=1.0,
                                           in1=g2[:, 128:256], op0=MULT, op1=MULT,
                                           accum_out=rowsum)
            fro2p = pp0.tile([1, 1], f32)
            nc.tensor.matmul(fro2p, lhsT=rowsum,
                             rhs=ones_col, start=True, stop=True)
            fro2_sb = p0.tile([1, 1], f32)
            nc.scalar.copy(fro2_sb, fro2p)
            frob = pp0.tile([128, 1], f32)
            nc.tensor.matmul(frob, lhsT=ones_row,
                             rhs=fro2_sb, start=True, stop=True)
            # sqrt(x) = exp(0.5*ln(x)): stays in the natural_log_exp table set and
            # avoids the low-precision Sqrt spline.
            lnf = p0.tile([128, 1], f32)
            nc.scalar.activation(lnf, frob, LN)
            fro_sb = p0.tile([128, 1], f32)
            nc.scalar.activation(fro_sb, lnf, EXP, scale=0.5)
            froe = p0.tile([128, 1], f32)
            nc.vector.tensor_scalar_add(froe, fro_sb, 1e-5)
            rs_sb = p0.tile([128, 1], f32)
            nc.vector.reciprocal(rs_sb, froe)

            nc.vector.memset(tsx_sb, 0.0)
            nc.vector.tensor_scalar_mul(
                tsx_sb[:, :, 0:128],
                tps.rearrange("p (h d) -> p h d", h=HPC),
                rs_sb,
            )

        # ---------------- per-head attention ----------------
        TG = cfg[0]
        bv = cfg[3] if len(cfg) > 3 else 0
        sps_bufs = {1: 4, 2: 2, 3: 2}[TG]
        pvs_bufs, oms_bufs = (3, 1) if bv == 1 else (2, 2)
        with tc.tile_pool(name="kq", bufs=2) as kq, \
             tc.tile_pool(name="vxp", bufs=2) as vxp, \
             tc.tile_pool(name="ep", bufs=cfg[4] if len(cfg) > 4 else 3) as ep, \
             tc.tile_pool(name="ob", bufs=cfg[6] if len(cfg) > 6 else 3) as ob, \
             tc.tile_pool(name="sm", bufs=cfg[8] if len(cfg) > 8 else 4) as sm, \
             tc.tile_pool(name="sps", bufs=sps_bufs, space="PSUM") as sps, \
             tc.tile_pool(name="pvs", bufs=pvs_bufs, space="PSUM") as pvs, \
             tc.tile_pool(name="oms", bufs=oms_bufs, space="PSUM") as oms:
            TG, om_in_pv = cfg[0], cfg[1]
            for h in range(HPC):
                tv = tvs[h]
                s_dt = f32r if cfg[2] == 'f32r' else bf16
                kt_sb = kq.tile([128, L], s_dt, tag="kt")
                nc.sync.dma_start(out=kt_sb[:, 0:tv * 128], in_=kt_d[h][:, 0:tv * 128])
                qt_sb = kq.tile([128, L], s_dt, tag="qt")
                nc.sync.dma_start(out=qt_sb, in_=qt_d[h])
                vx_sb = vxp.tile([128, NT, PADC], bf16, tag="vx")
                nc.sync.dma_start(
                    out=vx_sb[:, 0:tv, :],
                    in_=vx_d[h].rearrange("p (t c) -> p t c", t=NT)[:, 0:tv, :])

                for jb in range(NJB):
                    # two PSUM banks, each packing two q-subtile accumulators
                    # (cols [0:PADC] and [256:256+PADC]); col 128 of each region is
                    # the softmax denominator (ones column of vx).
                    pv0 = pvs.tile([128, 512], f32, tag="pv", name="pv0")
                    pv1 = pvs.tile([128, 512], f32, tag="pv", name="pv1")
                    pvb = [pv0, pv1]
                    pv_first = [None, None]

                    for tg in range((tv + TG - 1) // TG):
                        tls = [i for i in range(TG) if TG * tg + i < tv]
                        width = 512 * len(tls)
                        sp_t = sps.tile([128, 512 * TG], f32, tag="sp", name="sp_t")
                        for tl in tls:
                            t = TG * tg + tl
                            nc.tensor.matmul(
                                sp_t[:, tl * 512:(tl + 1) * 512],
                                lhsT=kt_sb[:, t * 128:(t + 1) * 128],
                                rhs=qt_sb[:, jb * 512:(jb + 1) * 512],
                                start=True, stop=True)
                        exps = ep.tile([128, 512 * TG], bf16, tag="exps", name="exps")
                        if len(cfg) > 7 and cfg[7]:
                            for tl in tls:
                                nc.scalar.activation(
                                    exps[:, tl * 512:(tl + 1) * 512],
                                    sp_t[:, tl * 512:(tl + 1) * 512], EXP,
                                    scale=scales_sb[:, h:h + 1])
                        else:
                            nc.scalar.activation(exps[:, 0:width], sp_t[:, 0:width],
                                                 EXP, scale=scales_sb[:, h:h + 1])
                        for tl in tls:
                            t = TG * tg + tl
                            for qs in range(4):
                                bank = pvb[qs // 2]
                                off = (qs % 2) * 256
                                # has_written packing: the first matmul emitted into a
                                # bank uses start=True (clears the whole bank's bits);
                                # the other region's t=0 matmul relies on its bits being
                                # clear -> plain write, then accumulates for t>=1.
                                st = (t == 0 and qs % 2 == 0)
                                mm = nc.tensor.matmul(
                                    bank[:, off:off + PADC],
                                    lhsT=exps[:, tl * 512 + qs * 128:
                                              tl * 512 + (qs + 1) * 128],
                                    rhs=vx_sb[:, t, :],
                                    start=st,
                                    stop=(t == tv - 1 and qs % 2 == 1),
                                    skip_group_check=True)
                                if t == 0:
                                    if qs % 2 == 0:
                                        pv_first[qs // 2] = mm
                                    else:
                                        add_dep_helper(
                                            mm.ins, pv_first[qs // 2].ins,
                                            sync=False,
                                            reason="psum has_written bank packing")

                    # epilogue part 1: rec + t1 per qs
                    t1s = []
                    for qs in range(4):
                        bank = pvb[qs // 2]
                        off = (qs % 2) * 256
                        rec = sm.tile([128, 1], f32, tag="rec", name="rec")
                        nc.vector.reciprocal(rec, bank[:, off + 128:off + 129])
                        t1 = sm.tile([128, 128], f32, tag="t1", name="t1")
                        nc.vector.tensor_scalar_mul(t1, bank[:, off:off + 128], rec)
                        t1s.append(t1)

                    # mult-term: out_m = q_mult @ (kv_mult^T V_h / fro)
                    if om_in_pv:
                        om_tiles = [(pv0, qs * 128) for qs in range(4)]
                    else:
                        om0 = oms.tile([128, 512], f32, tag="om", name="om0")
                        om_tiles = [(om0, qs * 128) for qs in range(4)]
                    om_first = None
                    for qs in range(4):
                        bank, off = om_tiles[qs]
                        mm = nc.tensor.matmul(
                            bank[:, off:off + 128],
                            lhsT=qmt_sb[:, jb * 512 + qs * 128:
                                        jb * 512 + (qs + 1) * 128],
                            rhs=tsx_sb[:, h, 0:128],
                            start=(qs == 0), stop=(qs == 3),
                            skip_group_check=True)
                        if qs == 0:
                            om_first = mm
                        else:
                            add_dep_helper(mm.ins, om_first.ins, sync=False,
                                           reason="psum has_written bank packing")

                    osb = ob.tile([128, 4, 128], f32, tag="osb", name="osb")
                    for qs in range(4):
                        bank, off = om_tiles[qs]
                        nc.vector.tensor_add(
                            osb[:, qs, :], t1s[qs], bank[:, off:off + 128])
                    nc.sync.dma_start(
                        out=out_d[h, jb * 512:(jb + 1) * 512, :]
                            .rearrange("(s p) d -> p s d", p=128),
                        in_=osb)


def build_program(repeat: int = 1, tvs=(NT,) * HPC, cfg=(2, False, 'f32r', 0)):
    nc = bacc.Bacc("TRN2", target_bir_lowering=False, debug=False,
                   enable_asserts=False, num_devices=NCORES)
    s_dt = f32r if cfg[2] == 'f32r' else bf16
    kt_d = nc.dram_tensor("kt", (HPC, 128, L), s_dt, kind="ExternalInput").ap()
    qt_d = nc.dram_tensor("qt", (HPC, 128, L), s_dt, kind="ExternalInput").ap()
    vx_d = nc.dram_tensor("vx", (HPC, 128, NT * PADC), bf16, kind="ExternalInput").ap()
    v4_d = nc.dram_tensor("v4", (128, NT * 512), f32r, kind="ExternalInput").ap()
    cm_d = nc.dram_tensor("cm", (128, NT * 256), f32r, kind="ExternalInput").ap()
    qmt_d = nc.dram_tensor("qmt", (128, L), bf16, kind="ExternalInput").ap()
    sc_d = nc.dram_tensor("sc", (128, HPC), f32, kind="ExternalInput").ap()
    out_d = nc.dram_tensor("out", (HPC, L, D), f32, kind="ExternalOutput").ap()

    with tile.TileContext(nc) as tc:
        for _ in range(repeat):
            _build_body(nc, tc, kt_d, qt_d, vx_d, v4_d, cm_d, qmt_d, sc_d, out_d, tvs, cfg)
    nc.compile()
    return nc


def head_order_and_tvs(valid_lens):
    vl = np.asarray(valid_lens).astype(np.int64)
    tv_all = np.where(vl == 0, NT, -(-vl // 128)).astype(int).reshape(NCORES, HPC)
    order = np.argsort(-tv_all, axis=1, kind="stable")
    sorted_tv = -np.sort(-tv_all, axis=1)
    tvs = tuple(int(x) for x in sorted_tv.max(axis=0))
    return order, tvs


def host_prepare(queries, keys, values, q_mult, kv_mult, valid_lens, num_heads,
                 order=None, s_bf16=False):
    queries = np.asarray(queries, dtype=np.float32)
    keys = np.asarray(keys, dtype=np.float32)
    values = np.asarray(values, dtype=np.float32)
    q_mult = np.asarray(q_mult, dtype=np.float32)
    kv_mult = np.asarray(kv_mult, dtype=np.float32)
    valid_lens = np.asarray(valid_lens).astype(np.int64)

    if order is None:
        order = np.tile(np.arange(HPC), (NCORES, 1))
    in_maps = []
    for c in range(NCORES):
        s = HPC * c
        b = s // num_heads
        idx = [s + int(order[c][i]) for i in range(HPC)]
        kt = np.ascontiguousarray(keys[idx].transpose(0, 2, 1))
        qt = np.ascontiguousarray(queries[idx].transpose(0, 2, 1))
        if s_bf16:
            kt = kt.astype(ml_dtypes.bfloat16)
            qt = qt.astype(ml_dtypes.bfloat16)

        vx = np.zeros((HPC, L, PADC), np.float32)
        sc = np.zeros((128, HPC), np.float32)
        for i in range(HPC):
            v = int(valid_lens[idx[i]])
            if v == 0:
                vx[i, :, 0:D] = values[idx[i]]
                vx[i, :, D] = 1.0
                sc[:, i] = 0.0
            else:
                m = (np.arange(L) < v).astype(np.float32)
                vx[i, :, 0:D] = values[idx[i]] * m[:, None]
                vx[i, :, D] = m
                sc[:, i] = 1.0 / np.sqrt(float(D))
        vxr = np.ascontiguousarray(
            vx.reshape(HPC, NT, 128, PADC).transpose(0, 2, 1, 3)
              .reshape(HPC, 128, NT * PADC)).astype(ml_dtypes.bfloat16)

        v4 = np.ascontiguousarray(
            values[idx].reshape(HPC, NT, 128, D).transpose(2, 1, 0, 3)
                  .reshape(128, NT * HPC * D))

        cmf = np.concatenate([q_mult[b], kv_mult[b]], axis=1)          # [L, 256]
        cm = np.ascontiguousarray(
            cmf.reshape(NT, 128, 256).transpose(1, 0, 2).reshape(128, NT * 256))
        qmt = np.ascontiguousarray(q_mult[b].T).astype(ml_dtypes.bfloat16)

        in_maps.append(dict(kt=kt, qt=qt, vx=vxr, v4=v4, cm=cm, qmt=qmt, sc=sc))
    return in_maps


_PROGRAM_CACHE = {}


DEFAULT_CFG = (2, False, 'f32r', 0, 5)   # (TG, om-in-pv, S dtype, bufs variant, ep bufs)


def _get_program(repeat: int = 1, tvs=(NT,) * HPC, cfg=None):
    cfg = DEFAULT_CFG if cfg is None else cfg
    key = (repeat, tuple(tvs), cfg)
    if key not in _PROGRAM_CACHE:
        _PROGRAM_CACHE[key] = build_program(repeat, tvs, cfg)
    return _PROGRAM_CACHE[key]


def kernel(queries, keys, values, q_mult, kv_mult, valid_lens, num_heads, **_unused):
    num_heads = int(np.asarray(num_heads))
    order, tvs = head_order_and_tvs(valid_lens)
    in_maps = host_prepare(queries, keys, values, q_mult, kv_mult, valid_lens,
                           num_heads, order)
    nc = _get_program(1, tvs)
    res = None
    for attempt in range(3):
        try:
            res = bass_utils.run_bass_kernel_spmd(
                nc, in_maps, core_ids=list(range(NCORES)))
            break
        except Exception:
            if attempt == 2:
                raise
            import time as _time
            _time.sleep(5)
    out = np.empty((NCORES * HPC, L, D), np.float32)
    for c in range(NCORES):
        o = np.asarray(res.results[c]["out"], np.float32)
        for i in range(HPC):
            out[HPC * c + int(order[c][i])] = o[i]
    return out


# revision 18
# speedup vs baseline: 1.2025x; 1.0307x over previous
"""Trainium2 Bass kernel for nn_DotProductAttention_50010599194781.

Computes, per (batch*head) bh:
    S = Q @ K^T / sqrt(d)                       [L, L]
    P = softmax(mask(S))                         (mask: key index >= valid_lens[bh] -> -1e6)
    mult = (q_mult[b] @ kv_mult[b]^T) / (||.||_F + 1e-5)   (per batch b, repeated over heads)
    out = (P + mult) @ V

Sharding: BH=32 heads split 4-per-core across 8 NeuronCores (SPMD, one program).

Device algorithm (per core, batch b fixed):
  - mult term via associativity:  mult @ V_h = q_mult @ (kv_mult^T @ V_h) / fro
    and  fro^2 = sum(G_q * G_k)  with  G_q = q_mult^T q_mult,  G_k = kv_mult^T kv_mult.
  - softmax part computed in transposed layout S^T[k, q] so exp(S^T) tiles are directly
    usable as matmul stationary weights for P @ V (no on-chip transposes).
  - masking is pure host-side data: V is pre-masked per head and an appended ones-column
    (also masked) yields the softmax denominator for free inside the P@V matmul.
    valid_len == 0 heads get scale=0 (exp->1 uniform) + unmasked V, matching jax softmax.
"""

import numpy as np
import ml_dtypes

import concourse.bass as bass
import concourse.tile as tile
from concourse import bacc, mybir, bass_utils
from concourse.tile_rust import add_dep_helper

B, H, L, D = 2, 16, 2048, 128
NCORES = 8
HPC = 4            # heads per core
NT = L // 128      # 16 k-tiles of 128
NJB = 4            # q-blocks of 512
PADC = 132         # PV rhs cols: 128 V + 1 ones + 3 pad (bf16)

f32 = mybir.dt.float32
f32r = mybir.dt.float32r
bf16 = mybir.dt.bfloat16

MULT = mybir.AluOpType.mult
ADD = mybir.AluOpType.add
EXP = mybir.ActivationFunctionType.Exp
LN = mybir.ActivationFunctionType.Ln


def _build_body(nc, tc, kt_d, qt_d, vx_d, v4_d, cm_d, qmt_d, sc_d, out_d, tvs, cfg):
    with tc.tile_pool(name="pers", bufs=1) as pers:
        tsx_sb = pers.tile([128, HPC, 256], bf16)
        bv0 = cfg[3] if len(cfg) > 3 else 0
        omall_sb = pers.tile([128, NT, HPC * 128], f32, name="omall_sb") if bv0 == 3 else None
        scales_sb = pers.tile([128, HPC], f32)
        qmt_sb = pers.tile([128, L], bf16)
        nc.sync.dma_start(out=scales_sb, in_=sc_d)
        nc.sync.dma_start(out=qmt_sb, in_=qmt_d)
        ones_col = pers.tile([128, 1], f32)
        nc.vector.memset(ones_col, 1.0)
        ones_row = pers.tile([1, 128], f32)
        nc.vector.memset(ones_row, 1.0)

        # ---------------- phase 0: mult-term prep (shared by the 4 heads) ---------
        with tc.tile_pool(name="p0", bufs=1) as p0, \
             tc.tile_pool(name="pp0", bufs=1, space="PSUM") as pp0:
            cm_sb = p0.tile([128, NT, 256], f32r)
            nc.sync.dma_start(out=cm_sb, in_=cm_d.rearrange("p (t c) -> p t c", t=NT))
            v4_sb = p0.tile([128, NT, 512], f32r)
            nc.sync.dma_start(out=v4_sb, in_=v4_d.rearrange("p (t c) -> p t c", t=NT))

            g1 = pp0.tile([128, 256], f32)    # q_mult^T @ [q_mult | kv_mult]
            g2 = pp0.tile([128, 256], f32)    # kv_mult^T @ [q_mult | kv_mult]
            tps = pp0.tile([128, 512], f32)   # kv_mult^T @ [V_h0 .. V_h3]
            for t in range(NT):
                nc.tensor.matmul(g1, lhsT=cm_sb[:, t, 0:128],
                                 rhs=cm_sb[:, t, :],
                                 start=(t == 0), stop=(t == NT - 1))
            for t in range(NT):
                nc.tensor.matmul(g2, lhsT=cm_sb[:, t, 128:256],
                                 rhs=cm_sb[:, t, :],
                                 start=(t == 0), stop=(t == NT - 1))
            for t in range(NT):
                nc.tensor.matmul(tps, lhsT=cm_sb[:, t, 128:256],
                                 rhs=v4_sb[:, t, :],
                                 start=(t == 0), stop=(t == NT - 1))

            # fro^2 = sum(G_q * G_k); DVE reads one PSUM operand per op.
            gq_sb = p0.tile([128, 128], f32)
            nc.vector.tensor_copy(gq_sb, g1[:, 0:128])
            prod = p0.tile([128, 128], f32)
            rowsum = p0.tile([128, 1], f32)
            nc.vector.scalar_tensor_tensor(out=prod, in0=gq_sb, scalar=1.0,
                                           in1=g2[:, 128:256], op0=MULT, op1=MULT,
                                           accum_out=rowsum)
            fro2p = pp0.tile([1, 1], f32)
            nc.tensor.matmul(fro2p, lhsT=rowsum,
                             rhs=ones_col, start=True, stop=True)
            fro2_sb = p0.tile([1, 1], f32)
            nc.scalar.copy(fro2_sb, fro2p)
            frob = pp0.tile([128, 1], f32)
            nc.tensor.matmul(frob, lhsT=ones_row,
                             rhs=fro2_sb, start=True, stop=True)
            # sqrt(x) = exp(0.5*ln(x)): stays in the natural_log_exp table set and
            # avoids the low-precision Sqrt spline.
            lnf = p0.tile([128, 1], f32)
            nc.scalar.activation(lnf, frob, LN)
            fro_sb = p0.tile([128, 1], f32)
            nc.scalar.activation(fro_sb, lnf, EXP, scale=0.5)
            froe = p0.tile([128, 1], f32)
            nc.vector.tensor_scalar_add(froe, fro_sb, 1e-5)
            rs_sb = p0.tile([128, 1], f32)
            nc.vector.reciprocal(rs_sb, froe)

            nc.vector.memset(tsx_sb, 0.0)
            nc.vector.tensor_scalar_mul(
                tsx_sb[:, :, 0:128],
                tps.rearrange("p (h d) -> p h d", h=HPC),
                rs_sb,
            )

            if (cfg[3] if len(cfg) > 3 else 0) == 3:
                # hoisted mult term: out_m[q, (h,d)] for all 4 heads at once,
                # one N=512 matmul per 128-row q-subtile, copied to SBUF so the
                # per-tile epilogue is a single fused (pv*rec)+om op.
                tsx_flat = pers.tile([128, HPC * 128], bf16, name="tsx_flat")
                nc.vector.tensor_copy(
                    tsx_flat.rearrange("p (h d) -> p h d", h=HPC),
                    tsx_sb[:, :, 0:128])
                for qg in range(NT):
                    om_ps = pp0.tile([128, 512], f32, tag="omps", bufs=2,
                                     name="om_ps")
                    nc.tensor.matmul(
                        om_ps,
                        lhsT=qmt_sb[:, qg * 128:(qg + 1) * 128],
                        rhs=tsx_flat,
                        start=True, stop=True)
                    nc.vector.tensor_copy(omall_sb[:, qg, :], om_ps)

        # ---------------- per-head attention ----------------
        TG = cfg[0]
        bv = cfg[3] if len(cfg) > 3 else 0
        sps_bufs = {1: 4, 2: 2, 3: 2}[TG]
        pvs_bufs, oms_bufs = {0: (2, 2), 1: (3, 1), 2: (4, 1), 3: (2, 1)}[bv]
        if bv == 3:
            sps_bufs = 3
        with tc.tile_pool(name="kq", bufs=cfg[5] if len(cfg) > 5 else 2) as kq, \
             tc.tile_pool(name="vxp", bufs=2) as vxp, \
             tc.tile_pool(name="ep", bufs=cfg[4] if len(cfg) > 4 else 3) as ep, \
             tc.tile_pool(name="ob", bufs=cfg[6] if len(cfg) > 6 else 3) as ob, \
             tc.tile_pool(name="sm", bufs=cfg[8] if len(cfg) > 8 else 4) as sm, \
             tc.tile_pool(name="sps", bufs=sps_bufs, space="PSUM") as sps, \
             tc.tile_pool(name="pvs", bufs=pvs_bufs, space="PSUM") as pvs, \
             tc.tile_pool(name="oms", bufs=oms_bufs, space="PSUM") as oms:
            TG, om_in_pv = cfg[0], cfg[1]
            for h in range(HPC):
                tv = tvs[h]
                s_dt = f32r if cfg[2] == 'f32r' else bf16
                kt_sb = kq.tile([128, L], s_dt, tag="kt")
                nc.sync.dma_start(out=kt_sb[:, 0:tv * 128], in_=kt_d[h][:, 0:tv * 128])
                qt_sb = kq.tile([128, L], s_dt, tag="qt")
                nc.sync.dma_start(out=qt_sb, in_=qt_d[h])
                vx_sb = vxp.tile([128, NT, PADC], bf16, tag="vx")
                nc.sync.dma_start(
                    out=vx_sb[:, 0:tv, :],
                    in_=vx_d[h].rearrange("p (t c) -> p t c", t=NT)[:, 0:tv, :])

                for jb in range(NJB):
                    # two PSUM banks, each packing two q-subtile accumulators
                    # (cols [0:PADC] and [256:256+PADC]); col 128 of each region is
                    # the softmax denominator (ones column of vx).
                    pv0 = pvs.tile([128, 512], f32, tag="pv", name="pv0")
                    pv1 = pvs.tile([128, 512], f32, tag="pv", name="pv1")
                    pvb = [pv0, pv1]
                    pv_first = [None, None]

                    for tg in range((tv + TG - 1) // TG):
                        tls = [i for i in range(TG) if TG * tg + i < tv]
                        width = 512 * len(tls)
                        sp_t = sps.tile([128, 512 * TG], f32, tag="sp", name="sp_t")
                        for tl in tls:
                            t = TG * tg + tl
                            nc.tensor.matmul(
                                sp_t[:, tl * 512:(tl + 1) * 512],
                                lhsT=kt_sb[:, t * 128:(t + 1) * 128],
                                rhs=qt_sb[:, jb * 512:(jb + 1) * 512],
                                start=True, stop=True)
                        exps = ep.tile([128, 512 * TG], bf16, tag="exps", name="exps")
                        if len(cfg) > 7 and cfg[7]:
                            for tl in tls:
                                nc.scalar.activation(
                                    exps[:, tl * 512:(tl + 1) * 512],
                                    sp_t[:, tl * 512:(tl + 1) * 512], EXP,
                                    scale=scales_sb[:, h:h + 1])
                        else:
                            nc.scalar.activation(exps[:, 0:width], sp_t[:, 0:width],
                                                 EXP, scale=scales_sb[:, h:h + 1])
                        for tl in tls:
                            t = TG * tg + tl
                            for qs in range(4):
                                bank = pvb[qs // 2]
                                off = (qs % 2) * 256
                                # has_written packing: the first matmul emitted into a
                                # bank uses start=True (clears the whole bank's bits);
                                # the other region's t=0 matmul relies on its bits being
                                # clear -> plain write, then accumulates for t>=1.
                                st = (t == 0 and qs % 2 == 0)
                                mm = nc.tensor.matmul(
                                    bank[:, off:off + PADC],
                                    lhsT=exps[:, tl * 512 + qs * 128:
                                              tl * 512 + (qs + 1) * 128],
                                    rhs=vx_sb[:, t, :],
                                    start=st,
                                    stop=(t == tv - 1 and qs % 2 == 1),
                                    skip_group_check=True)
                                if t == 0:
                                    if qs % 2 == 0:
                                        pv_first[qs // 2] = mm
                                    else:
                                        add_dep_helper(
                                            mm.ins, pv_first[qs // 2].ins,
                                            sync=False,
                                            reason="psum has_written bank packing")

                    # epilogue part 1: one merged reciprocal per bank ([128,2]
                    # over the two denominator columns), then t1 per qs.
                    recs = []
                    for bank in pvb:
                        rec2 = sm.tile([128, 2], f32, tag="rec", name="rec2")
                        nc.vector.reciprocal(
                            rec2, bank.rearrange("p (r c) -> p r c", r=2)[:, :, 128])
                        recs.append(rec2)
                    t1s = []
                    if bv != 3:
                        for qs in range(4):
                            bank = pvb[qs // 2]
                            off = (qs % 2) * 256
                            t1 = sm.tile([128, 128], f32, tag="t1", name="t1")
                            nc.vector.tensor_scalar_mul(
                                t1, bank[:, off:off + 128],
                                recs[qs // 2][:, (qs % 2):(qs % 2) + 1])
                            t1s.append(t1)

                    if bv == 3:
                        osb = ob.tile([128, 4, 128], f32, tag="osb", name="osb")
                        for qs in range(4):
                            bank = pvb[qs // 2]
                            off = (qs % 2) * 256
                            nc.vector.scalar_tensor_tensor(
                                out=osb[:, qs, :],
                                in0=bank[:, off:off + 128],
                                scalar=recs[qs // 2][:, (qs % 2):(qs % 2) + 1],
                                in1=omall_sb[:, jb * 4 + qs, h * 128:(h + 1) * 128],
                                op0=MULT, op1=ADD)
                        nc.sync.dma_start(
                            out=out_d[h, jb * 512:(jb + 1) * 512, :]
                                .rearrange("(s p) d -> p s d", p=128),
                            in_=osb)
                        continue

                    # mult-term: out_m = q_mult @ (kv_mult^T V_h / fro)
                    if om_in_pv:
                        om_tiles = [(pv0, qs * 128) for qs in range(4)]
                    else:
                        om0 = oms.tile([128, 512], f32, tag="om", name="om0")
                        om_tiles = [(om0, qs * 128) for qs in range(4)]
                    om_first = None
                    for qs in range(4):
                        bank, off = om_tiles[qs]
                        mm = nc.tensor.matmul(
                            bank[:, off:off + 128],
                            lhsT=qmt_sb[:, jb * 512 + qs * 128:
                                        jb * 512 + (qs + 1) * 128],
                            rhs=tsx_sb[:, h, 0:128],
                            start=(qs == 0), stop=(qs == 3),
                            skip_group_check=True)
                        if qs == 0:
                            om_first = mm
                        else:
                            add_dep_helper(mm.ins, om_first.ins, sync=False,
                                           reason="psum has_written bank packing")

                    osb = ob.tile([128, 4, 128], f32, tag="osb", name="osb")
                    for qs in range(4):
                        bank, off = om_tiles[qs]
                        nc.vector.tensor_add(
                            osb[:, qs, :], t1s[qs], bank[:, off:off + 128])
                    nc.sync.dma_start(
                        out=out_d[h, jb * 512:(jb + 1) * 512, :]
                            .rearrange("(s p) d -> p s d", p=128),
                        in_=osb)


def build_program(repeat: int = 1, tvs=(NT,) * HPC, cfg=(2, False, 'f32r', 0)):
    nc = bacc.Bacc("TRN2", target_bir_lowering=False, debug=False,
                   enable_asserts=False, num_devices=NCORES)
    s_dt = f32r if cfg[2] == 'f32r' else bf16
    kt_d = nc.dram_tensor("kt", (HPC, 128, L), s_dt, kind="ExternalInput").ap()
    qt_d = nc.dram_tensor("qt", (HPC, 128, L), s_dt, kind="ExternalInput").ap()
    vx_d = nc.dram_tensor("vx", (HPC, 128, NT * PADC), bf16, kind="ExternalInput").ap()
    v4_d = nc.dram_tensor("v4", (128, NT * 512), f32r, kind="ExternalInput").ap()
    cm_d = nc.dram_tensor("cm", (128, NT * 256), f32r, kind="ExternalInput").ap()
    qmt_d = nc.dram_tensor("qmt", (128, L), bf16, kind="ExternalInput").ap()
    sc_d = nc.dram_tensor("sc", (128, HPC), f32, kind="ExternalInput").ap()
    out_d = nc.dram_tensor("out", (HPC, L, D), f32, kind="ExternalOutput").ap()

    with tile.TileContext(nc) as tc:
        for _ in range(repeat):
            _build_body(nc, tc, kt_d, qt_d, vx_d, v4_d, cm_d, qmt_d, sc_d, out_d, tvs, cfg)
    nc.compile()
    return nc


def head_order_and_tvs(valid_lens):
    vl = np.asarray(valid_lens).astype(np.int64)
    tv_all = np.where(vl == 0, NT, -(-vl // 128)).astype(int).reshape(NCORES, HPC)
    order = np.argsort(-tv_all, axis=1, kind="stable")
    sorted_tv = -np.sort(-tv_all, axis=1)
    tvs = tuple(int(x) for x in sorted_tv.max(axis=0))
    return order, tvs


def host_prepare(queries, keys, values, q_mult, kv_mult, valid_lens, num_heads,
                 order=None, s_bf16=False):
    queries = np.asarray(queries, dtype=np.float32)
    keys = np.asarray(keys, dtype=np.float32)
    values = np.asarray(values, dtype=np.float32)
    q_mult = np.asarray(q_mult, dtype=np.float32)
    kv_mult = np.asarray(kv_mult, dtype=np.float32)
    valid_lens = np.asarray(valid_lens).astype(np.int64)

    if order is None:
        order = np.tile(np.arange(HPC), (NCORES, 1))
    in_maps = []
    for c in range(NCORES):
        s = HPC * c
        b = s // num_heads
        idx = [s + int(order[c][i]) for i in range(HPC)]
        kt = np.ascontiguousarray(keys[idx].transpose(0, 2, 1))
        qt = np.ascontiguousarray(queries[idx].transpose(0, 2, 1))
        if s_bf16:
            kt = kt.astype(ml_dtypes.bfloat16)
            qt = qt.astype(ml_dtypes.bfloat16)

        vx = np.zeros((HPC, L, PADC), np.float32)
        sc = np.zeros((128, HPC), np.float32)
        for i in range(HPC):
            v = int(valid_lens[idx[i]])
            if v == 0:
                vx[i, :, 0:D] = values[idx[i]]
                vx[i, :, D] = 1.0
                sc[:, i] = 0.0
            else:
                m = (np.arange(L) < v).astype(np.float32)
                vx[i, :, 0:D] = values[idx[i]] * m[:, None]
                vx[i, :, D] = m
                sc[:, i] = 1.0 / np.sqrt(float(D))
        vxr = np.ascontiguousarray(
            vx.reshape(HPC, NT, 128, PADC).transpose(0, 2, 1, 3)
              .reshape(HPC, 128, NT * PADC)).astype(ml_dtypes.bfloat16)

        v4 = np.ascontiguousarray(
            values[idx].reshape(HPC, NT, 128, D).transpose(2, 1, 0, 3)
                  .reshape(128, NT * HPC * D))

        cmf = np.concatenate([q_mult[b], kv_mult[b]], axis=1)          # [L, 256]
        cm = np.ascontiguousarray(
            cmf.reshape(NT, 128, 256).transpose(1, 0, 2).reshape(128, NT * 256))
        qmt = np.ascontiguousarray(q_mult[b].T).astype(ml_dtypes.bfloat16)

        in_maps.append(dict(kt=kt, qt=qt, vx=vxr, v4=v4, cm=cm, qmt=qmt, sc=sc))
    return in_maps


_PROGRAM_CACHE = {}


DEFAULT_CFG = (2, False, 'f32r', 3, 5)   # (TG, om-in-pv, S dtype, bufs variant, ep bufs); bv=3 = hoisted mult-term + fused epilogue + sps=3


def _get_program(repeat: int = 1, tvs=(NT,) * HPC, cfg=None):
    cfg = DEFAULT_CFG if cfg is None else cfg
    key = (repeat, tuple(tvs), cfg)
    if key not in _PROGRAM_CACHE:
        _PROGRAM_CACHE[key] = build_program(repeat, tvs, cfg)
    return _PROGRAM_CACHE[key]


def kernel(queries, keys, values, q_mult, kv_mult, valid_lens, num_heads, **_unused):
    num_heads = int(np.asarray(num_heads))
    order, tvs = head_order_and_tvs(valid_lens)
    in_maps = host_prepare(queries, keys, values, q_mult, kv_mult, valid_lens,
                           num_heads, order)
    nc = _get_program(1, tvs)
    res = None
    for attempt in range(3):
        try:
            res = bass_utils.run_bass_kernel_spmd(
                nc, in_maps, core_ids=list(range(NCORES)))
            break
        except Exception:
            if attempt == 2:
                raise
            import time as _time
            _time.sleep(5)
    out = np.empty((NCORES * HPC, L, D), np.float32)
    for c in range(NCORES):
        o = np.asarray(res.results[c]["out"], np.float32)
        for i in range(HPC):
            out[HPC * c + int(order[c][i])] = o[i]
    return out
